# revision 25
# baseline (speedup 1.0000x reference)
"""Trainium2 Bass kernel for DigitCapsuleLayer dynamic routing.

Strategy: data-parallel over batch (32 per core x 8 cores). The routing is
computed in a fully factored form that never materializes u_hat
[B,1152,10,16]:

  q[b,c,m,i] = sum_g  cij[(g,m),c] * u[b,(g,m),i]      (PE, block-diag cij)
  s[b,c,o]   = sum_mi W[m,c,o,i]   * q[b,c,m,i]        (PE, after a DVE
                                                        32x32 block transpose
                                                        moves i to partitions)
  v = squash(s)                                        (PE ones-trick + ACT
                                                        ln/exp form: fsc =
                                                        exp(.5 ln msq -
                                                            ln(1+msq)))
  p[b,c,m,i] = sum_o  W[m,c,o,i]   * v[b,c,o]          (PE, block-diag v)
  a[r,c]     = sum_bi u[b,r,i]/B   * p[b,c,m,i]        (PE, u_t stationary)
  AllReduce(a) across 8 cores; b_ij += a; cij = softmax(b_ij)

All matmul operands are bf16 (PSUM accumulate in fp32): 1 PE cycle/row vs 4
for fp32. The squash ln/exp form keeps every ACT op inside the single
natural_log_exp_and_others table (no 1283ns ACT table swaps, no slow DVE
reciprocal on [128,320]).

Indices: r = g*32+m (g<36, m<32), m = 2t+m_sub (t<16), i = h*4+i4 (h<2,i4<4).
Row spaces: q-contraction rows = ms*64+g (u_qp/cij_bd; 36..63/100..127 zero);
b_ij/cij/a rows compact = ms*36+g (72 rows); q/p rows = i4*32+b;
s/v rows = rep*32+o (o<16, 4x replicated).
"""

import sys
import numpy as np
import ml_dtypes

sys.path.insert(0, "/opt/trn_rl_repo")
sys.path.insert(0, "/opt/trn_rl_repo/concourse")

BF16 = ml_dtypes.bfloat16
NC_CORES = 8
_ACT_JSON = None


def _install_act_tables():
    """Reorder act_info.json so 'natural_log_exp_and_others' is the first
    set: the lowering picks the first set containing each activation func,
    so Ln/Exp/Square/Copy all share one table -> no 1283ns table reloads."""
    global _ACT_JSON
    import os
    import json
    import tempfile
    if _ACT_JSON is not None or os.environ.get("KERNEL_NO_ACT_OVERRIDE"):
        return
    from neuronxcc.driver.Job import Job
    from neuronxcc.driver.jobs.support.FindActInfo import findActInfoFile
    src = findActInfoFile(Job.getPackageDir(), "gen3")
    with open(src) as f:
        info = json.load(f)
    sets = info["act_func_sets"]
    sets.sort(key=lambda e: e["name"] != "natural_log_exp_and_others")
    fd, path = tempfile.mkstemp(suffix="_act_info.json")
    with os.fdopen(fd, "w") as f:
        json.dump(info, f)
    os.environ["BASS_ACT_ROOT_JSON_PATH"] = path
    _ACT_JSON = path
BL = 32           # batch per core
G, M32, C, O, I = 36, 32, 10, 16, 8
T16 = 16


# ----------------------------------------------------------------- host prep
def _host_prep(u, W):
    """u [256,1152,8] f32, W [32,10,16,8] f32 -> per-core input maps."""
    u = np.ascontiguousarray(u, np.float32)
    W = np.ascontiguousarray(W, np.float32)

    # u_qp [core, 128, t, h, 128]; row p = m_sub*64+g ; col = i4*32+b
    u8 = u.reshape(NC_CORES, BL, G, T16, 2, 2, 4)   # [n, b, g, t, ms, h, i4]
    perm = u8.transpose(0, 2, 4, 3, 5, 6, 1)        # [n, g, ms, t, h, i4, b]
    u_qp = np.zeros((NC_CORES, 128, T16, 2, 128), np.float32)
    u_qp_v = u_qp.reshape(NC_CORES, 128, T16, 2, 4, 32)
    for ms in range(2):
        u_qp_v[:, ms * 64:ms * 64 + G] = perm[:, :, ms]

    # u_a3 [core, k, 128=(i4,b), t, 128=(ms*64+g)]: prescaled by 1/256;
    # cols 36..63 / 100..127 zero so the a-matmul lands ms blocks 64-aligned
    ua = u.reshape(NC_CORES, BL, G, T16, 2, 2, 4)   # [n, b, g, t, ms, k, i4]
    uat = ua.transpose(0, 5, 6, 1, 3, 4, 2)         # [n, k, i4, b, t, ms, g]
    u_a3 = np.zeros((NC_CORES, 2, 4, BL, T16, 2, 64), np.float32)
    u_a3[..., :G] = uat * np.float32(1.0 / 256.0)
    u_a3 = u_a3.reshape(NC_CORES, 2, 128, T16, 128)

    # w_s4 [k, 128, c, 128]: row i4*32+m, col (rep, half, o): half0 = W[m,c,o,i],
    # half1 = 0 pad so matmul M=128 lands v at partitions rep*32+o.
    wi = W.transpose(3, 0, 1, 2)                    # [i, m, c, o]
    w_s4 = np.zeros((2, 4, M32, C, 4, 2, O), np.float32)
    w_s4[:, :, :, :, :, 0, :] = wi.reshape(2, 4, M32, C, 1, O)
    w_s4 = np.ascontiguousarray(w_s4.reshape(2, 128, C, 128))

    # w_p2 [128, c, blk, m]: row i4*32+o (o<16; +16..31 zero) = W[m,c,o,blk*4+i4]
    wp = W.transpose(3, 2, 1, 0)                    # [i, o, c, m]
    w_p2 = np.zeros((4, 2, O, C, 2, M32), np.float32)
    w_p2[:, 0] = wp.reshape(2, 4, O, C, M32).transpose(1, 2, 3, 0, 4)
    w_p2 = np.ascontiguousarray(w_p2.reshape(128, C, 2, M32))

    # ones_bd [128, 128]: 16x16 ones blocks at (i4*32..+16, i4*32..+16)
    ones_bd = np.zeros((4, 2, O, 4, 2, O), np.float32)
    for j in range(4):
        ones_bd[j, 0, :, j, 0, :] = 1.0
    ones_bd = np.ascontiguousarray(ones_bd.reshape(128, 128))

    u_qp = u_qp.astype(BF16)
    u_a3 = u_a3.astype(BF16)
    w_s4 = w_s4.astype(BF16)
    w_p2 = w_p2.astype(BF16)
    ones_bd = ones_bd.astype(BF16)

    in_maps = []
    for c in range(NC_CORES):
        in_maps.append({
            "u_qp": u_qp[c],
            "u_a3": u_a3[c],
            "w_s4": w_s4,
            "w_p2": w_p2,
            "ones_bd": ones_bd,
        })
    return in_maps


# ------------------------------------------------------------- bass builder
def _build_nc():
    from contextlib import ExitStack
    import concourse.bacc as bacc
    import concourse.tile as tile
    from concourse import mybir

    f32 = mybir.dt.float32
    bf16 = mybir.dt.bfloat16
    Act = mybir.ActivationFunctionType
    nc = bacc.Bacc("TRN2", target_bir_lowering=False, debug=False,
                   num_devices=NC_CORES)

    u_qp_p = nc.dram_tensor("u_qp", [128, T16, 2, 128], bf16, kind="ExternalInput")
    u_a3_p = nc.dram_tensor("u_a3", [2, 128, T16, 128], bf16, kind="ExternalInput")
    w_s4_p = nc.dram_tensor("w_s4", [2, 128, C, 128], bf16, kind="ExternalInput")
    w_p2_p = nc.dram_tensor("w_p2", [128, C, 2, M32], bf16, kind="ExternalInput")
    ones_p = nc.dram_tensor("ones_bd", [128, 128], bf16, kind="ExternalInput")
    v_out_p = nc.dram_tensor("v_out", [16, C, BL], f32, kind="ExternalOutput")

    rg = [list(range(NC_CORES))]

    with tile.TileContext(nc) as tc, ExitStack() as ctx:
        sb = ctx.enter_context(tc.tile_pool(name="sb", bufs=1))
        ps = ctx.enter_context(tc.tile_pool(name="ps", bufs=1, space="PSUM"))
        dr = ctx.enter_context(tc.tile_pool(name="dr", bufs=1, space="DRAM"))

        # --- persistent SBUF tiles ---
        u_qp = sb.tile([128, T16, 2, 128], bf16, tag="u_qp")
        u_a3 = [sb.tile([128, T16, 128], bf16, tag=f"u_a3_{k}", name=f"u_a3_{k}") for k in range(2)]
        w_s4 = [sb.tile([128, C, 128], bf16, tag=f"w_s4_{k}", name=f"w_s4_{k}") for k in range(2)]
        w_p2 = sb.tile([128, C, 2, M32], bf16, tag="w_p2")
        ones = sb.tile([128, 128], bf16, tag="ones")
        b_ij = sb.tile([G, T16, 2, C], f32, tag="b_ij")
        cij_bd = sb.tile([128, T16, 2, C], bf16, tag="cij_bd")
        v_bd = sb.tile([128, C, 4, BL], bf16, tag="v_bd")

        for tchunk in range(4):
            nc.sync.dma_start(out=u_qp[:, 4 * tchunk:4 * (tchunk + 1)],
                              in_=u_qp_p[:, 4 * tchunk:4 * (tchunk + 1)])
        for k in range(2):
            nc.sync.dma_start(out=u_a3[k][:], in_=u_a3_p[k])
            nc.sync.dma_start(out=w_s4[k][:], in_=w_s4_p[k])
        nc.sync.dma_start(out=w_p2[:], in_=w_p2_p[:])
        nc.sync.dma_start(out=ones[:], in_=ones_p[:])
        nc.vector.memset(b_ij[:], 0.0)
        nc.vector.memset(cij_bd[:].bitcast(f32), 0.0)
        nc.vector.memset(v_bd[:].bitcast(f32), 0.0)
        eps_c = sb.tile([128, 1], f32, tag="eps_c")
        nc.vector.memset(eps_c[:], 1e-20)

        # --- PSUM tiles (persist across iterations) ---
        q_psum = [ps.tile([128, T16, 2, C], f32, tag=f"q_ps{h}", name=f"q_ps{h}") for h in range(2)]
        s_psum = ps.tile([128, C, BL], f32, tag="s_ps")
        mag_ps = ps.tile([128, C, BL], f32, tag="mag_ps")
        p_psum = ps.tile([128, C, 2, M32], f32, tag="p_ps")
        a_psum = ps.tile([128, T16, C, 2], f32, tag="a_ps")

        for it in range(3):
            last = it == 2
            if it == 0:
                # b_ij = 0 -> cij = 0.1 exactly; write the diagonal directly
                for ms in range(2):
                    nc.vector.memset(cij_bd[ms * 64:ms * 64 + G, :, ms, :], 0.1)
            else:
                # ---- softmax over capsules (no max-shift; logits bounded) ----
                eb = sb.tile([G, T16, 2, C], f32, tag="eb")
                ssum = sb.tile([G, T16, 2], f32, tag="ssum")
                rs = sb.tile([G, T16, 2], f32, tag="rs")
                nc.scalar.activation(eb[:], b_ij[:], Act.Exp)
                nc.vector.tensor_reduce(ssum[:], eb[:], axis=mybir.AxisListType.X,
                                        op=mybir.AluOpType.add)
                nc.vector.reciprocal(rs[:], ssum[:])
                for ms, eng in ((0, nc.vector), (1, nc.gpsimd)):
                    eng.tensor_mul(
                        cij_bd[ms * 64:ms * 64 + G, :, ms, :],
                        eb[:, :, ms, :],
                        rs[:, :, ms, None].broadcast_to((G, T16, C)))

            # ---- q: per (t, h) matmul, K=128 over (m_sub, g) ----
            for h in range(2):
                for t in range(T16):
                    nc.tensor.matmul(q_psum[h][:, t], lhsT=u_qp[:, t, h, :],
                                     rhs=cij_bd[:, t, :, :], start=True, stop=True)
            # cast PSUM->bf16 (ACT), then transpose 32x32 blocks on DVE; input
            # AP enumerates (c, m): col c*32 + t*2 + ms <- offset t*20 + ms*10 + c
            q_c = [sb.tile([128, T16, 2, C], bf16, tag=f"q_c{h}", name=f"q_c{h}") for h in range(2)]
            q_T = [sb.tile([128, C, M32], bf16, tag=f"q_T{h}", name=f"q_T{h}") for h in range(2)]
            for h in range(2):
                nc.scalar.copy(q_c[h][:], q_psum[h][:])
                src_r = q_c[h][:].rearrange("p t ms c -> p c (t ms)")
                nc.vector.transpose(out=q_T[h][:], in_=src_r)

            # ---- s: per (c, k) accumulate; out rows = (rep, o) 4x-replicated ----
            for c10 in range(C):
                for k in range(2):
                    nc.tensor.matmul(s_psum[:, c10, :], lhsT=w_s4[k][:, c10, :],
                                     rhs=q_T[k][:, c10, :],
                                     start=(k == 0), stop=(k == 1))

            # ---- squash on [64=(rep,o), c, b]: fsc = exp(.5 ln m - ln(1+m)) ----
            s2 = sb.tile([128, C, BL], bf16, tag="s2")
            a_ln = sb.tile([128, C, BL], f32, tag="a_ln")
            b_ln = sb.tile([128, C, BL], f32, tag="b_ln")
            fsc = sb.tile([128, C, BL], f32, tag="fsc")
            v_rep = sb.tile([16, C, BL], bf16, tag="v_rep")
            v_out_sb = sb.tile([16, C, BL], f32, tag="v_out_sb")
            p_sb = sb.tile([128, C, 2, M32], bf16, tag="p_sb")
            # ACT ops batched per act-table: Square (exp set), then all Ln
            # (natural_log set), then all Exp -> 2 table loads per iteration
            CG = [slice(0, 5), slice(5, C)]
            for cg in CG:
                nc.scalar.activation(s2[:, cg, :], s_psum[:, cg, :], Act.Square)
                nc.tensor.matmul(mag_ps[:, cg, :], lhsT=ones[:], rhs=s2[:, cg, :],
                                 start=True, stop=True)
            for cg in CG:
                # pad rows have msq=0: the 1e-20 bias keeps ln finite there
                nc.scalar.activation(a_ln[:, cg, :], mag_ps[:, cg, :], Act.Ln,
                                     bias=eps_c[:])
                nc.scalar.activation(b_ln[:, cg, :], mag_ps[:, cg, :], Act.Ln,
                                     bias=1.0)
                nc.vector.scalar_tensor_tensor(out=fsc[:, cg, :],
                                               in0=a_ln[:, cg, :], scalar=0.5,
                                               in1=b_ln[:, cg, :],
                                               op0=mybir.AluOpType.mult,
                                               op1=mybir.AluOpType.subtract)
            for cg in CG:
                nc.scalar.activation(fsc[:, cg, :], fsc[:, cg, :], Act.Exp)
                if last:
                    nc.vector.tensor_mul(v_out_sb[:, cg, :], s_psum[0:16, cg, :],
                                         fsc[0:16, cg, :])
                    continue
                # v once on DVE (bf16), then fan out to the 4 block-diag
                # slots on the Pool engine (SBUF->SBUF)
                nc.vector.tensor_mul(v_rep[:, cg, :], s_psum[0:16, cg, :],
                                     fsc[0:16, cg, :])
                for j in range(4):
                    nc.gpsimd.tensor_copy(out=v_bd[j * 32:j * 32 + 16, cg, j, :],
                                          in_=v_rep[:, cg, :])
                for c10 in range(C)[cg]:
                    nc.tensor.matmul(p_psum[:, c10, :, :],
                                     lhsT=v_bd[:, c10, :, :],
                                     rhs=w_p2[:, c10, :, :],
                                     start=True, stop=True)

            if last:
                nc.sync.dma_start(out=v_out_p[:], in_=v_out_sb[:])
                break

            nc.scalar.copy(p_sb[:, 0:5], p_psum[:, 0:5])
            nc.scalar.copy(p_sb[:, 5:C], p_psum[:, 5:C])

            # ---- a: per (t, k) accumulate; out rows = (ms*36+g), cols (c, ms) ----
            for t in range(T16):
                for k in range(2):
                    nc.tensor.matmul(a_psum[:, t, :, :],
                                     lhsT=u_a3[k][:, t, :],
                                     rhs=p_sb[:, :, k, 2 * t:2 * t + 2],
                                     start=(k == 0), stop=(k == 1))
            a_sb = sb.tile([G, T16, 2, C], bf16, tag="a_sb")
            a_red = sb.tile([G, T16, 2, C], bf16, tag="a_red")
            for ms in range(2):
                nc.vector.tensor_copy(out=a_sb[:, :, ms, :],
                                      in_=a_psum[ms * 64:ms * 64 + G, :, :, ms])

            cc_in = dr.tile([G, T16 * 2 * C], bf16, tag=f"cc_in{it}", name=f"cc_in{it}")
            cc_out = dr.tile([G, T16 * 2 * C], bf16, tag=f"cc_out{it}",
                             name=f"cc_out{it}", addr_space="Shared")
            nc.sync.dma_start(out=cc_in[:], in_=a_sb[:])
            nc.gpsimd.collective_compute(
                "AllReduce", mybir.AluOpType.add, replica_groups=rg,
                ins=[cc_in[:].opt()], outs=[cc_out[:].opt()])
            nc.sync.dma_start(out=a_red[:], in_=cc_out[:])
            nc.vector.tensor_add(b_ij[:], b_ij[:], a_red[:])

    nc.finalize()
    return nc


_NC_CACHE = None


def kernel(u, W):
    """u [256,1152,8] f32, W [32,10,16,8] f32 -> [256,10,16,1] f32."""
    global _NC_CACHE
    from concourse import bass_utils

    in_maps = _host_prep(u, W)
    if _NC_CACHE is None:
        _NC_CACHE = _build_nc()
    res = bass_utils.run_bass_kernel_spmd(
        _NC_CACHE, in_maps, core_ids=list(range(NC_CORES)))

    out = np.zeros((NC_CORES * BL, C, O, 1), np.float32)
    for c in range(NC_CORES):
        vo = res.results[c]["v_out"]          # [16, C, BL] = [o, c, b]
        out[c * BL:(c + 1) * BL, :, :, 0] = vo.transpose(2, 1, 0)
    return out


if __name__ == "__main__":
    u = np.random.randn(256, 1152, 8).astype(np.float32)
    W = np.random.randn(32, 10, 16, 8).astype(np.float32)
    v = kernel(u, W)
    print("kernel ran, out shape", v.shape, "absmax", np.abs(v).max())


# revision 30
# speedup vs baseline: 1.4292x; 1.4292x over previous
"""Trainium2 Bass kernel for DigitCapsuleLayer dynamic routing.

Strategy: data-parallel over batch (32 per core x 8 cores). The routing is
computed in a fully factored form that never materializes u_hat
[B,1152,10,16]:

  q[b,c,m,i] = sum_g  cij[(g,m),c] * u[b,(g,m),i]      (PE, block-diag cij)
  s[b,c,o]   = sum_mi W[m,c,o,i]   * q[b,c,m,i]        (PE, after a DVE
                                                        32x32 block transpose
                                                        moves i to partitions)
  v = squash(s)                                        (PE ones-trick + ACT
                                                        ln/exp form: fsc =
                                                        exp(.5 ln msq -
                                                            ln(1+msq)))
  p[b,c,m,i] = sum_o  W[m,c,o,i]   * v[b,c,o]          (PE, block-diag v)
  a[r,c]     = sum_bi u[b,r,i]/B   * p[b,c,m,i]        (PE, u_t stationary)
  AllReduce(a) across 8 cores; b_ij += a; cij = softmax(b_ij)

All matmul operands are bf16 (PSUM accumulate in fp32): 1 PE cycle/row vs 4
for fp32. The squash ln/exp form keeps every ACT op inside the single
natural_log_exp_and_others table (no 1283ns ACT table swaps, no slow DVE
reciprocal on [128,320]).

Indices: r = g*32+m (g<36, m<32), m = 2t+m_sub (t<16), i = h*4+i4 (h<2,i4<4).
Row spaces: q-contraction rows = ms*64+g (u_qp/cij_bd; 36..63/100..127 zero);
b_ij/cij/a rows compact = ms*36+g (72 rows); q/p rows = i4*32+b;
s/v rows = rep*32+o (o<16, 4x replicated).
"""

import sys
import numpy as np
import ml_dtypes

sys.path.insert(0, "/opt/trn_rl_repo")
sys.path.insert(0, "/opt/trn_rl_repo/concourse")

BF16 = ml_dtypes.bfloat16
NC_CORES = 8
_ACT_JSON = None


def _install_act_tables():
    """Reorder act_info.json so 'natural_log_exp_and_others' is the first
    set: the lowering picks the first set containing each activation func,
    so Ln/Exp/Square/Copy all share one table -> no 1283ns table reloads."""
    global _ACT_JSON
    import os
    import json
    import tempfile
    if _ACT_JSON is not None or os.environ.get("KERNEL_NO_ACT_OVERRIDE"):
        return
    from neuronxcc.driver.Job import Job
    from neuronxcc.driver.jobs.support.FindActInfo import findActInfoFile
    src = findActInfoFile(Job.getPackageDir(), "gen3")
    with open(src) as f:
        info = json.load(f)
    sets = info["act_func_sets"]
    sets.sort(key=lambda e: e["name"] != "natural_log_exp_and_others")
    fd, path = tempfile.mkstemp(suffix="_act_info.json")
    with os.fdopen(fd, "w") as f:
        json.dump(info, f)
    os.environ["BASS_ACT_ROOT_JSON_PATH"] = path
    _ACT_JSON = path
BL = 32           # batch per core
G, M32, C, O, I = 36, 32, 10, 16, 8
T16 = 16


# ----------------------------------------------------------------- host prep
def _host_prep(u, W):
    """u [256,1152,8] f32, W [32,10,16,8] f32 -> per-core input maps."""
    u = np.ascontiguousarray(u, np.float32)
    W = np.ascontiguousarray(W, np.float32)

    # u_qp [core, 128, t, h, 128]; row p = m_sub*64+g ; col = i4*32+b
    u8 = u.reshape(NC_CORES, BL, G, T16, 2, 2, 4)   # [n, b, g, t, ms, h, i4]
    perm = u8.transpose(0, 2, 4, 3, 5, 6, 1)        # [n, g, ms, t, h, i4, b]
    u_qp = np.zeros((NC_CORES, 128, T16, 2, 128), np.float32)
    u_qp_v = u_qp.reshape(NC_CORES, 128, T16, 2, 4, 32)
    for ms in range(2):
        u_qp_v[:, ms * 64:ms * 64 + G] = perm[:, :, ms]

    # u_a3 [core, k, 128=(i4,b), t, 128=(ms*64+g)]: prescaled by 1/256;
    # cols 36..63 / 100..127 zero so the a-matmul lands ms blocks 64-aligned
    ua = u.reshape(NC_CORES, BL, G, T16, 2, 2, 4)   # [n, b, g, t, ms, k, i4]
    uat = ua.transpose(0, 5, 6, 1, 3, 4, 2)         # [n, k, i4, b, t, ms, g]
    u_a3 = np.zeros((NC_CORES, 2, 4, BL, T16, 2, 64), np.float32)
    u_a3[..., :G] = uat * np.float32(1.0 / 256.0)
    u_a3 = u_a3.reshape(NC_CORES, 2, 128, T16, 128)

    # w_s4 [k, 128, c, 128]: row i4*32+m, col (rep, half, o): half0 = W[m,c,o,i],
    # half1 = 0 pad so matmul M=128 lands v at partitions rep*32+o.
    wi = W.transpose(3, 0, 1, 2)                    # [i, m, c, o]
    w_s4 = np.zeros((2, 4, M32, C, 4, 2, O), np.float32)
    w_s4[:, :, :, :, :, 0, :] = wi.reshape(2, 4, M32, C, 1, O)
    w_s4 = np.ascontiguousarray(w_s4.reshape(2, 128, C, 128))

    # w_p2 [128, c, blk, m]: row i4*32+o (o<16; +16..31 zero) = W[m,c,o,blk*4+i4]
    wp = W.transpose(3, 2, 1, 0)                    # [i, o, c, m]
    w_p2 = np.zeros((4, 2, O, C, 2, M32), np.float32)
    w_p2[:, 0] = wp.reshape(2, 4, O, C, M32).transpose(1, 2, 3, 0, 4)
    w_p2 = np.ascontiguousarray(w_p2.reshape(128, C, 2, M32))

    # ones_bd [128, 128]: 16x16 ones blocks at (i4*32..+16, i4*32..+16)
    ones_bd = np.zeros((4, 2, O, 4, 2, O), np.float32)
    for j in range(4):
        ones_bd[j, 0, :, j, 0, :] = 1.0
    ones_bd = np.ascontiguousarray(ones_bd.reshape(128, 128))

    u_qp = u_qp.astype(BF16)
    u_a3 = u_a3.astype(BF16)
    w_s4 = w_s4.astype(BF16)
    w_p2 = w_p2.astype(BF16)
    ones_bd = ones_bd.astype(BF16)

    in_maps = []
    for c in range(NC_CORES):
        in_maps.append({
            "u_qp": u_qp[c],
            "u_a3": u_a3[c],
            "w_s4": w_s4,
            "w_p2": w_p2,
            "ones_bd": ones_bd,
        })
    return in_maps


# ------------------------------------------------------------- bass builder
def _build_nc():
    from contextlib import ExitStack
    import concourse.bacc as bacc
    import concourse.tile as tile
    from concourse import mybir

    f32 = mybir.dt.float32
    bf16 = mybir.dt.bfloat16
    Act = mybir.ActivationFunctionType
    nc = bacc.Bacc("TRN2", target_bir_lowering=False, debug=False,
                   num_devices=NC_CORES)

    u_qp_p = nc.dram_tensor("u_qp", [128, T16, 2, 128], bf16, kind="ExternalInput")
    u_a3_p = nc.dram_tensor("u_a3", [2, 128, T16, 128], bf16, kind="ExternalInput")
    w_s4_p = nc.dram_tensor("w_s4", [2, 128, C, 128], bf16, kind="ExternalInput")
    w_p2_p = nc.dram_tensor("w_p2", [128, C, 2, M32], bf16, kind="ExternalInput")
    ones_p = nc.dram_tensor("ones_bd", [128, 128], bf16, kind="ExternalInput")
    v_out_p = nc.dram_tensor("v_out", [16, C, BL], f32, kind="ExternalOutput")

    rg = [list(range(NC_CORES))]

    with tile.TileContext(nc) as tc, ExitStack() as ctx:
        sb = ctx.enter_context(tc.tile_pool(name="sb", bufs=1))
        ps = ctx.enter_context(tc.tile_pool(name="ps", bufs=1, space="PSUM"))
        dr = ctx.enter_context(tc.tile_pool(name="dr", bufs=1, space="DRAM"))

        # --- persistent SBUF tiles ---
        u_qp = sb.tile([128, T16, 2, 128], bf16, tag="u_qp")
        u_a3 = [sb.tile([128, T16, 128], bf16, tag=f"u_a3_{k}", name=f"u_a3_{k}") for k in range(2)]
        w_s4 = [sb.tile([128, C, 128], bf16, tag=f"w_s4_{k}", name=f"w_s4_{k}") for k in range(2)]
        w_p2 = sb.tile([128, C, 2, M32], bf16, tag="w_p2")
        ones = sb.tile([128, 128], bf16, tag="ones")
        b_ij = sb.tile([G, T16, 2, C], f32, tag="b_ij")
        cij_bd = sb.tile([128, T16, 2, C], bf16, tag="cij_bd")
        v_bd = sb.tile([128, C, 4, BL], bf16, tag="v_bd")

        for tchunk in range(4):
            nc.sync.dma_start(out=u_qp[:, 4 * tchunk:4 * (tchunk + 1)],
                              in_=u_qp_p[:, 4 * tchunk:4 * (tchunk + 1)])
        for k in range(2):
            nc.sync.dma_start(out=u_a3[k][:], in_=u_a3_p[k])
            nc.sync.dma_start(out=w_s4[k][:], in_=w_s4_p[k])
        nc.sync.dma_start(out=w_p2[:], in_=w_p2_p[:])
        nc.sync.dma_start(out=ones[:], in_=ones_p[:])
        nc.vector.memset(b_ij[:], 0.0)
        # warm-up collective with no input dependency: fires at t~0 so the
        # CC queues initialize while iter-0 compute runs (values unused)
        warm_in = dr.tile([1, 20], f32, tag="warm_in", name="warm_in")
        warm_out = dr.tile([1, 20], f32, tag="warm_out", name="warm_out",
                           addr_space="Shared")
        nc.gpsimd.collective_compute(
            "AllReduce", mybir.AluOpType.add, replica_groups=rg,
            ins=[warm_in[:].opt()], outs=[warm_out[:].opt()])
        nc.vector.memset(cij_bd[:].bitcast(f32), 0.0)
        nc.vector.memset(v_bd[:].bitcast(f32), 0.0)
        eps_c = sb.tile([128, 1], f32, tag="eps_c")
        nc.vector.memset(eps_c[:], 1e-20)
        lnpre = sb.tile([1, 1], f32, tag="lnpre")

        # --- PSUM tiles (persist across iterations) ---
        q_psum = [ps.tile([128, T16, 2, C], f32, tag=f"q_ps{h}", name=f"q_ps{h}") for h in range(2)]
        s_psum = ps.tile([128, C, BL], f32, tag="s_ps")
        mag_ps = ps.tile([128, C, BL], f32, tag="mag_ps")
        p_psum = ps.tile([128, C, 2, M32], f32, tag="p_ps")
        a_psum = ps.tile([128, T16, C, 2], f32, tag="a_ps")

        for it in range(3):
            last = it == 2
            if it == 0:
                # b_ij = 0 -> cij = 0.1 exactly; write the diagonal directly
                for ms in range(2):
                    nc.vector.memset(cij_bd[ms * 64:ms * 64 + G, :, ms, :], 0.1)
                # prefetch the natural_log ACT table during iter-0 q/s
                nc.scalar.activation(lnpre[:], cij_bd[0:1, 0:1, 0:1, 0:1],
                                     Act.Ln, bias=1.0)
            else:
                # ---- softmax over capsules (no max-shift; logits bounded) ----
                eb = sb.tile([G, T16, 2, C], f32, tag="eb")
                ssum = sb.tile([G, T16, 2], f32, tag="ssum")
                rs = sb.tile([G, T16, 2], f32, tag="rs")
                nc.scalar.activation(eb[:], b_ij[:], Act.Exp)
                nc.vector.tensor_reduce(ssum[:], eb[:], axis=mybir.AxisListType.X,
                                        op=mybir.AluOpType.add)
                nc.vector.reciprocal(rs[:], ssum[:])
                for ms in range(2):
                    nc.vector.tensor_mul(
                        cij_bd[ms * 64:ms * 64 + G, :, ms, :],
                        eb[:, :, ms, :],
                        rs[:, :, ms, None].broadcast_to((G, T16, C)))
                # prefetch the natural_log ACT table while the PE runs q/s;
                # input dep on eb pins this after the softmax Exp
                nc.scalar.activation(lnpre[:], eb[0:1, 0:1, 0:1, 0:1], Act.Ln,
                                     bias=1.0)

            # ---- q: per (t, h) matmul, K=128 over (m_sub, g) ----
            for h in range(2):
                for t in range(T16):
                    nc.tensor.matmul(q_psum[h][:, t], lhsT=u_qp[:, t, h, :],
                                     rhs=cij_bd[:, t, :, :], start=True, stop=True)
            # cast PSUM->bf16 (ACT), then transpose 32x32 blocks on DVE; input
            # AP enumerates (c, m): col c*32 + t*2 + ms <- offset t*20 + ms*10 + c
            q_c = [sb.tile([128, T16, 2, C], bf16, tag=f"q_c{h}", name=f"q_c{h}") for h in range(2)]
            q_T = [sb.tile([128, C, M32], bf16, tag=f"q_T{h}", name=f"q_T{h}") for h in range(2)]
            for h in range(2):
                nc.scalar.copy(q_c[h][:], q_psum[h][:])
                src_r = q_c[h][:].rearrange("p t ms c -> p c (t ms)")
                nc.vector.transpose(out=q_T[h][:], in_=src_r)

            # ---- s: per (c, k) accumulate; out rows = (rep, o) 4x-replicated ----
            for c10 in range(C):
                for k in range(2):
                    nc.tensor.matmul(s_psum[:, c10, :], lhsT=w_s4[k][:, c10, :],
                                     rhs=q_T[k][:, c10, :],
                                     start=(k == 0), stop=(k == 1))

            # ---- squash on [64=(rep,o), c, b]: fsc = exp(.5 ln m - ln(1+m)) ----
            s2 = sb.tile([128, C, BL], bf16, tag="s2")
            a_ln = sb.tile([128, C, BL], f32, tag="a_ln")
            b_ln = sb.tile([128, C, BL], f32, tag="b_ln")
            fsc = sb.tile([128, C, BL], f32, tag="fsc")
            v_rep = sb.tile([16, C, BL], bf16, tag="v_rep")
            v_out_sb = sb.tile([16, C, BL], f32, tag="v_out_sb")
            p_sb = sb.tile([128, C, 2, M32], bf16, tag="p_sb")
            # single-chain squash: data deps force the ACT order
            # square -> ln -> ln -> exp, so exactly the Ln and Exp table
            # loads remain (the Ln one prefetched by lnpre above)
            nc.scalar.activation(s2[:], s_psum[:], Act.Square)
            nc.tensor.matmul(mag_ps[:], lhsT=ones[:], rhs=s2[:],
                             start=True, stop=True)
            # pad rows have msq=0: the 1e-20 bias keeps ln finite there
            nc.scalar.activation(a_ln[:], mag_ps[:], Act.Ln, bias=eps_c[:])
            nc.scalar.activation(b_ln[:], mag_ps[:], Act.Ln, bias=1.0)
            nc.vector.scalar_tensor_tensor(out=fsc[:],
                                           in0=a_ln[:], scalar=0.5,
                                           in1=b_ln[:],
                                           op0=mybir.AluOpType.mult,
                                           op1=mybir.AluOpType.subtract)
            nc.scalar.activation(fsc[:], fsc[:], Act.Exp)
            if last:
                nc.vector.tensor_mul(v_out_sb[:], s_psum[0:16], fsc[0:16])
            else:
                # v once on DVE (bf16), fan out to the 4 block-diag slots
                # split across DVE and ACT
                nc.vector.tensor_mul(v_rep[:], s_psum[0:16], fsc[0:16])
                for j in range(2):
                    nc.vector.tensor_copy(out=v_bd[j * 32:j * 32 + 16, :, j, :],
                                          in_=v_rep[:])
                for j in range(2, 4):
                    nc.scalar.copy(v_bd[j * 32:j * 32 + 16, :, j, :], v_rep[:])
                for c10 in range(C):
                    nc.tensor.matmul(p_psum[:, c10, :, :],
                                     lhsT=v_bd[:, c10, :, :],
                                     rhs=w_p2[:, c10, :, :],
                                     start=True, stop=True)

            if last:
                nc.sync.dma_start(out=v_out_p[:], in_=v_out_sb[:])
                break

            nc.scalar.copy(p_sb[:, 0:5], p_psum[:, 0:5])
            nc.scalar.copy(p_sb[:, 5:C], p_psum[:, 5:C])

            # ---- a: per (t, k) accumulate; out rows = (ms*36+g), cols (c, ms) ----
            for t in range(T16):
                for k in range(2):
                    nc.tensor.matmul(a_psum[:, t, :, :],
                                     lhsT=u_a3[k][:, t, :],
                                     rhs=p_sb[:, :, k, 2 * t:2 * t + 2],
                                     start=(k == 0), stop=(k == 1))
            a_sb = sb.tile([G, T16, 2, C], bf16, tag="a_sb")
            a_red = sb.tile([G, T16, 2, C], bf16, tag="a_red")
            for ms in range(2):
                nc.vector.tensor_copy(out=a_sb[:, :, ms, :],
                                      in_=a_psum[ms * 64:ms * 64 + G, :, :, ms])

            cc_in = dr.tile([G, T16 * 2 * C], bf16, tag=f"cc_in{it}", name=f"cc_in{it}")
            cc_out = dr.tile([G, T16 * 2 * C], bf16, tag=f"cc_out{it}",
                             name=f"cc_out{it}", addr_space="Shared")
            nc.sync.dma_start(out=cc_in[:], in_=a_sb[:])
            nc.gpsimd.collective_compute(
                "AllReduce", mybir.AluOpType.add, replica_groups=rg,
                ins=[cc_in[:].opt()], outs=[cc_out[:].opt()])
            nc.sync.dma_start(out=a_red[:], in_=cc_out[:])
            nc.vector.tensor_add(b_ij[:], b_ij[:], a_red[:])

    nc.finalize()
    return nc


_NC_CACHE = None


def kernel(u, W):
    """u [256,1152,8] f32, W [32,10,16,8] f32 -> [256,10,16,1] f32."""
    global _NC_CACHE
    from concourse import bass_utils

    in_maps = _host_prep(u, W)
    if _NC_CACHE is None:
        _NC_CACHE = _build_nc()
    res = bass_utils.run_bass_kernel_spmd(
        _NC_CACHE, in_maps, core_ids=list(range(NC_CORES)))

    out = np.zeros((NC_CORES * BL, C, O, 1), np.float32)
    for c in range(NC_CORES):
        vo = res.results[c]["v_out"]          # [16, C, BL] = [o, c, b]
        out[c * BL:(c + 1) * BL, :, :, 0] = vo.transpose(2, 1, 0)
    return out


if __name__ == "__main__":
    u = np.random.randn(256, 1152, 8).astype(np.float32)
    W = np.random.randn(32, 10, 16, 8).astype(np.float32)
    v = kernel(u, W)
    print("kernel ran, out shape", v.shape, "absmax", np.abs(v).max())


# revision 34
# speedup vs baseline: 1.5438x; 1.0802x over previous
"""Trainium2 Bass kernel for DigitCapsuleLayer dynamic routing.

Strategy: data-parallel over batch (32 per core x 8 cores). The routing is
computed in a fully factored form that never materializes u_hat
[B,1152,10,16]:

  q[b,c,m,i] = sum_g  cij[(g,m),c] * u[b,(g,m),i]      (PE, block-diag cij)
  s[b,c,o]   = sum_mi W[m,c,o,i]   * q[b,c,m,i]        (PE, after a DVE
                                                        32x32 block transpose
                                                        moves i to partitions)
  v = squash(s)                                        (PE ones-trick + ACT
                                                        ln/exp form: fsc =
                                                        exp(.5 ln msq -
                                                            ln(1+msq)))
  p[b,c,m,i] = sum_o  W[m,c,o,i]   * v[b,c,o]          (PE, block-diag v)
  a[r,c]     = sum_bi u[b,r,i]/B   * p[b,c,m,i]        (PE, u_t stationary)
  AllReduce(a) across 8 cores; b_ij += a; cij = softmax(b_ij)

All matmul operands are bf16 (PSUM accumulate in fp32): 1 PE cycle/row vs 4
for fp32. The squash ln/exp form keeps every ACT op inside the single
natural_log_exp_and_others table (no 1283ns ACT table swaps, no slow DVE
reciprocal on [128,320]).

Indices: r = g*32+m (g<36, m<32), m = 2t+m_sub (t<16), i = h*4+i4 (h<2,i4<4).
Row spaces: q-contraction rows = ms*64+g (u_qp/cij_bd; 36..63/100..127 zero);
b_ij/cij/a rows compact = ms*36+g (72 rows); q/p rows = i4*32+b;
s/v rows = rep*32+o (o<16, 4x replicated).
"""

import sys
import numpy as np
import ml_dtypes

sys.path.insert(0, "/opt/trn_rl_repo")
sys.path.insert(0, "/opt/trn_rl_repo/concourse")

BF16 = ml_dtypes.bfloat16
NC_CORES = 8
BL = 32           # batch per core
G, M32, C, O, I = 36, 32, 10, 16, 8
T16 = 16


# ----------------------------------------------------------------- host prep
def _host_prep(u, W):
    """u [256,1152,8] f32, W [32,10,16,8] f32 -> per-core input maps."""
    u = np.ascontiguousarray(u, np.float32)
    W = np.ascontiguousarray(W, np.float32)

    # u_qp [core, 128, t, h, 128]; row p = m_sub*64+g ; col = i4*32+b
    u8 = u.reshape(NC_CORES, BL, G, T16, 2, 2, 4)   # [n, b, g, t, ms, h, i4]
    perm = u8.transpose(0, 2, 4, 3, 5, 6, 1)        # [n, g, ms, t, h, i4, b]
    u_qp = np.zeros((NC_CORES, 128, T16, 2, 128), np.float32)
    u_qp_v = u_qp.reshape(NC_CORES, 128, T16, 2, 4, 32)
    for ms in range(2):
        u_qp_v[:, ms * 64:ms * 64 + G] = perm[:, :, ms]

    # u_a3 [core, k, 128=(i4,b), t, 128=(ms*64+g)]: prescaled by 1/256;
    # cols 36..63 / 100..127 zero so the a-matmul lands ms blocks 64-aligned
    ua = u.reshape(NC_CORES, BL, G, T16, 2, 2, 4)   # [n, b, g, t, ms, k, i4]
    uat = ua.transpose(0, 5, 6, 1, 3, 4, 2)         # [n, k, i4, b, t, ms, g]
    u_a3 = np.zeros((NC_CORES, 2, 4, BL, T16, 2, 64), np.float32)
    u_a3[..., :G] = uat * np.float32(1.0 / 256.0)
    u_a3 = u_a3.reshape(NC_CORES, 2, 128, T16, 128)

    # w_s4 [k, 128, c, 128]: row i4*32+m, col (rep, half, o): half0 = W[m,c,o,i],
    # half1 = 0 pad so matmul M=128 lands v at partitions rep*32+o.
    wi = W.transpose(3, 0, 1, 2)                    # [i, m, c, o]
    w_s4 = np.zeros((2, 4, M32, C, 4, 2, O), np.float32)
    w_s4[:, :, :, :, :, 0, :] = wi.reshape(2, 4, M32, C, 1, O)
    w_s4 = np.ascontiguousarray(w_s4.reshape(2, 128, C, 128))

    # w_p2 [128, c, blk, m]: row i4*32+o (o<16; +16..31 zero) = W[m,c,o,blk*4+i4]
    wp = W.transpose(3, 2, 1, 0)                    # [i, o, c, m]
    w_p2 = np.zeros((4, 2, O, C, 2, M32), np.float32)
    w_p2[:, 0] = wp.reshape(2, 4, O, C, M32).transpose(1, 2, 3, 0, 4)
    w_p2 = np.ascontiguousarray(w_p2.reshape(128, C, 2, M32))

    # ones_bd [128, 128]: 16x16 ones blocks at (i4*32..+16, i4*32..+16)
    ones_bd = np.zeros((4, 2, O, 4, 2, O), np.float32)
    for j in range(4):
        ones_bd[j, 0, :, j, 0, :] = 1.0
    ones_bd = np.ascontiguousarray(ones_bd.reshape(128, 128))

    u_qp = u_qp.astype(BF16)
    u_a3 = u_a3.astype(BF16)
    w_s4 = w_s4.astype(BF16)
    w_p2 = w_p2.astype(BF16)
    ones_bd = ones_bd.astype(BF16)

    in_maps = []
    for c in range(NC_CORES):
        in_maps.append({
            "u_qp": u_qp[c],
            "u_a3": u_a3[c],
            "w_s4": w_s4,
            "w_p2": w_p2,
            "ones_bd": ones_bd,
        })
    return in_maps


# ------------------------------------------------------------- bass builder
def _build_nc():
    from contextlib import ExitStack
    import concourse.bacc as bacc
    import concourse.tile as tile
    from concourse import mybir

    f32 = mybir.dt.float32
    bf16 = mybir.dt.bfloat16
    Act = mybir.ActivationFunctionType
    nc = bacc.Bacc("TRN2", target_bir_lowering=False, debug=False,
                   num_devices=NC_CORES)

    u_qp_p = nc.dram_tensor("u_qp", [128, T16, 2, 128], bf16, kind="ExternalInput")
    u_a3_p = nc.dram_tensor("u_a3", [2, 128, T16, 128], bf16, kind="ExternalInput")
    w_s4_p = nc.dram_tensor("w_s4", [2, 128, C, 128], bf16, kind="ExternalInput")
    w_p2_p = nc.dram_tensor("w_p2", [128, C, 2, M32], bf16, kind="ExternalInput")
    ones_p = nc.dram_tensor("ones_bd", [128, 128], bf16, kind="ExternalInput")
    v_out_p = nc.dram_tensor("v_out", [16, C, BL], f32, kind="ExternalOutput")

    rg = [list(range(NC_CORES))]

    with tile.TileContext(nc) as tc, ExitStack() as ctx:
        sb = ctx.enter_context(tc.tile_pool(name="sb", bufs=1))
        ps = ctx.enter_context(tc.tile_pool(name="ps", bufs=1, space="PSUM"))
        dr = ctx.enter_context(tc.tile_pool(name="dr", bufs=1, space="DRAM"))

        # --- persistent SBUF tiles ---
        u_qp = sb.tile([128, T16, 2, 128], bf16, tag="u_qp")
        u_a3 = [sb.tile([128, T16, 128], bf16, tag=f"u_a3_{k}", name=f"u_a3_{k}") for k in range(2)]
        w_s4 = [sb.tile([128, C, 128], bf16, tag=f"w_s4_{k}", name=f"w_s4_{k}") for k in range(2)]
        w_p2 = sb.tile([128, C, 2, M32], bf16, tag="w_p2")
        ones = sb.tile([128, 128], bf16, tag="ones")
        b_ij = sb.tile([G, T16, 2, C], f32, tag="b_ij")
        cij_bd = sb.tile([128, T16, 2, C], bf16, tag="cij_bd")
        v_bd = sb.tile([128, C, 4, BL], bf16, tag="v_bd")

        for tchunk in range(4):
            nc.sync.dma_start(out=u_qp[:, 4 * tchunk:4 * (tchunk + 1)],
                              in_=u_qp_p[:, 4 * tchunk:4 * (tchunk + 1)])
        for k in range(2):
            nc.sync.dma_start(out=u_a3[k][:], in_=u_a3_p[k])
            nc.sync.dma_start(out=w_s4[k][:], in_=w_s4_p[k])
        nc.sync.dma_start(out=w_p2[:], in_=w_p2_p[:])
        nc.sync.dma_start(out=ones[:], in_=ones_p[:])
        nc.vector.memset(b_ij[:], 0.0)
        # warm-up collective with no input dependency: fires at t~0 so the
        # CC queues initialize while iter-0 compute runs (values unused)
        warm_in = dr.tile([1, 20], f32, tag="warm_in", name="warm_in")
        warm_out = dr.tile([1, 20], f32, tag="warm_out", name="warm_out",
                           addr_space="Shared")
        nc.gpsimd.collective_compute(
            "AllReduce", mybir.AluOpType.add, replica_groups=rg,
            ins=[warm_in[:].opt()], outs=[warm_out[:].opt()])
        nc.vector.memset(cij_bd[:].bitcast(f32), 0.0)
        nc.vector.memset(v_bd[:].bitcast(f32), 0.0)
        eps_c = sb.tile([128, 1], f32, tag="eps_c")
        nc.vector.memset(eps_c[:], 1e-20)
        lnpre = sb.tile([1, 1], f32, tag="lnpre")

        # --- PSUM tiles (persist across iterations) ---
        q_psum = [ps.tile([128, T16, 2, C], f32, tag=f"q_ps{h}", name=f"q_ps{h}") for h in range(2)]
        s_psum = ps.tile([128, C, BL], f32, tag="s_ps")
        mag_ps = ps.tile([128, C, BL], f32, tag="mag_ps")
        p_psum = ps.tile([128, C, 2, M32], f32, tag="p_ps")
        a_psum = ps.tile([128, T16, C, 2], f32, tag="a_ps")

        for it in range(3):
            last = it == 2
            if it == 0:
                # b_ij = 0 -> cij = 0.1 exactly; write the diagonal directly
                for ms in range(2):
                    nc.vector.memset(cij_bd[ms * 64:ms * 64 + G, :, ms, :], 0.1)
                # prefetch the natural_log ACT table during iter-0 q/s
                nc.scalar.activation(lnpre[:], cij_bd[0:1, 0:1, 0:1, 0:1],
                                     Act.Ln, bias=1.0)
            else:
                # ---- softmax over capsules (no max-shift; logits bounded) ----
                eb = sb.tile([G, T16, 2, C], f32, tag="eb")
                ssum = sb.tile([G, T16, 2], f32, tag="ssum")
                rs = sb.tile([G, T16, 2], f32, tag="rs")
                nc.scalar.activation(eb[:], b_ij[:], Act.Exp)
                nc.vector.tensor_reduce(ssum[:], eb[:], axis=mybir.AxisListType.X,
                                        op=mybir.AluOpType.add)
                nc.vector.reciprocal(rs[:], ssum[:])
                for ms in range(2):
                    nc.vector.tensor_mul(
                        cij_bd[ms * 64:ms * 64 + G, :, ms, :],
                        eb[:, :, ms, :],
                        rs[:, :, ms, None].broadcast_to((G, T16, C)))
                # prefetch the natural_log ACT table while the PE runs q/s;
                # input dep on eb pins this after the softmax Exp
                nc.scalar.activation(lnpre[:], eb[0:1, 0:1, 0:1, 0:1], Act.Ln,
                                     bias=1.0)

            # ---- q: per (t, h) matmul, K=128 over (m_sub, g) ----
            for h in range(2):
                for t in range(T16):
                    nc.tensor.matmul(q_psum[h][:, t], lhsT=u_qp[:, t, h, :],
                                     rhs=cij_bd[:, t, :, :], start=True, stop=True)
            # cast PSUM->bf16 (ACT), then transpose 32x32 blocks on DVE; input
            # AP enumerates (c, m): col c*32 + t*2 + ms <- offset t*20 + ms*10 + c
            q_c = [sb.tile([128, T16, 2, C], bf16, tag=f"q_c{h}", name=f"q_c{h}") for h in range(2)]
            q_T = [sb.tile([128, C, M32], bf16, tag=f"q_T{h}", name=f"q_T{h}") for h in range(2)]
            for h in range(2):
                nc.scalar.copy(q_c[h][:], q_psum[h][:])
                src_r = q_c[h][:].rearrange("p t ms c -> p c (t ms)")
                nc.vector.transpose(out=q_T[h][:], in_=src_r)

            # ---- s: per (c, k) accumulate; out rows = (rep, o) 4x-replicated ----
            for c10 in range(C):
                for k in range(2):
                    nc.tensor.matmul(s_psum[:, c10, :], lhsT=w_s4[k][:, c10, :],
                                     rhs=q_T[k][:, c10, :],
                                     start=(k == 0), stop=(k == 1))

            # ---- squash on [64=(rep,o), c, b]: fsc = exp(.5 ln m - ln(1+m)) ----
            s2 = sb.tile([128, C, BL], bf16, tag="s2")
            a_ln = sb.tile([128, C, BL], f32, tag="a_ln")
            b_ln = sb.tile([128, C, BL], f32, tag="b_ln")
            fsc = sb.tile([128, C, BL], f32, tag="fsc")
            v_rep = sb.tile([16, C, BL], bf16, tag="v_rep")
            v_out_sb = sb.tile([16, C, BL], f32, tag="v_out_sb")
            p_sb = sb.tile([128, C, 2, M32], bf16, tag="p_sb")
            # single-chain squash: data deps force the ACT order
            # square -> ln -> ln -> exp, so exactly the Ln and Exp table
            # loads remain (the Ln one prefetched by lnpre above)
            nc.scalar.activation(s2[:], s_psum[:], Act.Square)
            nc.tensor.matmul(mag_ps[:], lhsT=ones[:], rhs=s2[:],
                             start=True, stop=True)
            # pad rows have msq=0: the 1e-20 bias keeps ln finite there
            nc.scalar.activation(a_ln[:], mag_ps[:], Act.Ln, bias=eps_c[:])
            nc.scalar.activation(b_ln[:], mag_ps[:], Act.Ln, bias=1.0)
            # prefetch the Exp table while the DVE runs the stt below
            nc.scalar.activation(lnpre[:], b_ln[0:1, 0:1, 0:1], Act.Exp)
            nc.vector.scalar_tensor_tensor(out=fsc[:],
                                           in0=a_ln[:], scalar=0.5,
                                           in1=b_ln[:],
                                           op0=mybir.AluOpType.mult,
                                           op1=mybir.AluOpType.subtract)
            nc.scalar.activation(fsc[:], fsc[:], Act.Exp)
            if last:
                nc.vector.tensor_mul(v_out_sb[:], s_psum[0:16], fsc[0:16])
            else:
                # v once on DVE (bf16), fan out to the 4 block-diag slots
                # split across DVE and ACT
                nc.vector.tensor_mul(v_rep[:], s_psum[0:16], fsc[0:16])
                for j in range(2):
                    nc.vector.tensor_copy(out=v_bd[j * 32:j * 32 + 16, :, j, :],
                                          in_=v_rep[:])
                for j in range(2, 4):
                    nc.scalar.copy(v_bd[j * 32:j * 32 + 16, :, j, :], v_rep[:])
                for c10 in range(C):
                    nc.tensor.matmul(p_psum[:, c10, :, :],
                                     lhsT=v_bd[:, c10, :, :],
                                     rhs=w_p2[:, c10, :, :],
                                     start=True, stop=True)

            if last:
                nc.sync.dma_start(out=v_out_p[:], in_=v_out_sb[:])
                break

            # k-split casts (ACT + DVE in parallel); the k=0 a-matmul sweep
            # starts as soon as the k=0 cast lands
            nc.scalar.copy(p_sb[:, :, 0, :], p_psum[:, :, 0, :])
            nc.vector.tensor_copy(out=p_sb[:, :, 1, :], in_=p_psum[:, :, 1, :])

            # ---- a: per (t, k) accumulate; out rows = (ms*64+g), cols (c, ms) ----
            for t in range(T16):
                for k in range(2):
                    nc.tensor.matmul(a_psum[:, t, :, :],
                                     lhsT=u_a3[k][:, t, :],
                                     rhs=p_sb[:, :, k, 2 * t:2 * t + 2],
                                     start=(k == 0), stop=(k == 1))
            a_sb = sb.tile([G, T16, 2, C], bf16, tag="a_sb")
            a_red = sb.tile([G, T16, 2, C], bf16, tag="a_red")
            for ms in range(2):
                nc.vector.tensor_copy(out=a_sb[:, :, ms, :],
                                      in_=a_psum[ms * 64:ms * 64 + G, :, :, ms])

            cc_in = dr.tile([G, T16 * 2 * C], bf16, tag=f"cc_in{it}", name=f"cc_in{it}")
            cc_out = dr.tile([G, T16 * 2 * C], bf16, tag=f"cc_out{it}",
                             name=f"cc_out{it}", addr_space="Shared")
            nc.sync.dma_start(out=cc_in[:], in_=a_sb[:])
            nc.gpsimd.collective_compute(
                "AllReduce", mybir.AluOpType.add, replica_groups=rg,
                ins=[cc_in[:].opt()], outs=[cc_out[:].opt()])
            nc.sync.dma_start(out=a_red[:], in_=cc_out[:])
            nc.vector.tensor_add(b_ij[:], b_ij[:], a_red[:])

    nc.finalize()
    return nc


_NC_CACHE = None


def kernel(u, W):
    """u [256,1152,8] f32, W [32,10,16,8] f32 -> [256,10,16,1] f32."""
    global _NC_CACHE
    from concourse import bass_utils

    in_maps = _host_prep(u, W)
    if _NC_CACHE is None:
        _NC_CACHE = _build_nc()
    res = bass_utils.run_bass_kernel_spmd(
        _NC_CACHE, in_maps, core_ids=list(range(NC_CORES)))

    out = np.zeros((NC_CORES * BL, C, O, 1), np.float32)
    for c in range(NC_CORES):
        vo = res.results[c]["v_out"]          # [16, C, BL] = [o, c, b]
        out[c * BL:(c + 1) * BL, :, :, 0] = vo.transpose(2, 1, 0)
    return out


if __name__ == "__main__":
    u = np.random.randn(256, 1152, 8).astype(np.float32)
    W = np.random.randn(32, 10, 16, 8).astype(np.float32)
    v = kernel(u, W)
    print("kernel ran, out shape", v.shape, "absmax", np.abs(v).max())


# revision 35
# speedup vs baseline: 1.7383x; 1.1260x over previous
"""Trainium2 Bass kernel for DigitCapsuleLayer dynamic routing.

Strategy: data-parallel over batch (32 per core x 8 cores). The routing is
computed in a fully factored form that never materializes u_hat
[B,1152,10,16]:

  q[b,c,m,i] = sum_g  cij[(g,m),c] * u[b,(g,m),i]      (PE, block-diag cij)
  s[b,c,o]   = sum_mi W[m,c,o,i]   * q[b,c,m,i]        (PE, after a DVE
                                                        32x32 block transpose
                                                        moves i to partitions)
  v = squash(s)                                        (PE ones-trick + ACT
                                                        ln/exp form: fsc =
                                                        exp(.5 ln msq -
                                                            ln(1+msq)))
  p[b,c,m,i] = sum_o  W[m,c,o,i]   * v[b,c,o]          (PE, block-diag v)
  a[r,c]     = sum_bi u[b,r,i]/B   * p[b,c,m,i]        (PE, u_t stationary)
  AllReduce(a) across 8 cores; b_ij += a; cij = softmax(b_ij)

All matmul operands are bf16 (PSUM accumulate in fp32): 1 PE cycle/row vs 4
for fp32. The squash ln/exp form keeps every ACT op inside the single
natural_log_exp_and_others table (no 1283ns ACT table swaps, no slow DVE
reciprocal on [128,320]).

Indices: r = g*32+m (g<36, m<32), m = 2t+m_sub (t<16), i = h*4+i4 (h<2,i4<4).
Row spaces: q-contraction rows = ms*64+g (u_qp/cij_bd; 36..63/100..127 zero);
b_ij/cij/a rows compact = ms*36+g (72 rows); q/p rows = i4*32+b;
s/v rows = rep*32+o (o<16, 4x replicated).
"""

import sys
import numpy as np
import ml_dtypes

sys.path.insert(0, "/opt/trn_rl_repo")
sys.path.insert(0, "/opt/trn_rl_repo/concourse")

BF16 = ml_dtypes.bfloat16
NC_CORES = 8
BL = 32           # batch per core
G, M32, C, O, I = 36, 32, 10, 16, 8
T16 = 16


# ----------------------------------------------------------------- host prep
def _host_prep(u, W):
    """u [256,1152,8] f32, W [32,10,16,8] f32 -> per-core input maps."""
    u = np.ascontiguousarray(u, np.float32)
    W = np.ascontiguousarray(W, np.float32)

    # u_qp [core, 128, t, h, 128]; row p = m_sub*64+g ; col = i4*32+b
    u8 = u.reshape(NC_CORES, BL, G, T16, 2, 2, 4)   # [n, b, g, t, ms, h, i4]
    perm = u8.transpose(0, 2, 4, 3, 5, 6, 1)        # [n, g, ms, t, h, i4, b]
    u_qp = np.zeros((NC_CORES, 128, T16, 2, 128), np.float32)
    u_qp_v = u_qp.reshape(NC_CORES, 128, T16, 2, 4, 32)
    for ms in range(2):
        u_qp_v[:, ms * 64:ms * 64 + G] = perm[:, :, ms]

    # u_a3 [core, k, 128=(i4,b), t, 128=(ms*64+g)]: prescaled by 1/256;
    # cols 36..63 / 100..127 zero so the a-matmul lands ms blocks 64-aligned
    ua = u.reshape(NC_CORES, BL, G, T16, 2, 2, 4)   # [n, b, g, t, ms, k, i4]
    uat = ua.transpose(0, 5, 6, 1, 3, 4, 2)         # [n, k, i4, b, t, ms, g]
    u_a3 = np.zeros((NC_CORES, 2, 4, BL, T16, 2, 64), np.float32)
    u_a3[..., :G] = uat * np.float32(1.0 / 256.0)
    u_a3 = u_a3.reshape(NC_CORES, 2, 128, T16, 128)

    # w_s4 [k, 128, c, 128]: row i4*32+m, col (rep, half, o): half0 = W[m,c,o,i],
    # half1 = 0 pad so matmul M=128 lands v at partitions rep*32+o.
    wi = W.transpose(3, 0, 1, 2)                    # [i, m, c, o]
    w_s4 = np.zeros((2, 4, M32, C, 4, 2, O), np.float32)
    w_s4[:, :, :, :, :, 0, :] = wi.reshape(2, 4, M32, C, 1, O)
    w_s4 = np.ascontiguousarray(w_s4.reshape(2, 128, C, 128))

    # w_p2 [128, c, blk, m]: row i4*32+o (o<16; +16..31 zero) = W[m,c,o,blk*4+i4]
    wp = W.transpose(3, 2, 1, 0)                    # [i, o, c, m]
    w_p2 = np.zeros((4, 2, O, C, 2, M32), np.float32)
    w_p2[:, 0] = wp.reshape(2, 4, O, C, M32).transpose(1, 2, 3, 0, 4)
    w_p2 = np.ascontiguousarray(w_p2.reshape(128, C, 2, M32))

    # ones_bd [128, 128]: 16x16 ones blocks at (i4*32..+16, i4*32..+16)
    ones_bd = np.zeros((4, 2, O, 4, 2, O), np.float32)
    for j in range(4):
        ones_bd[j, 0, :, j, 0, :] = 1.0
    ones_bd = np.ascontiguousarray(ones_bd.reshape(128, 128))

    u_qp = u_qp.astype(BF16)
    u_a3 = u_a3.astype(BF16)
    w_s4 = w_s4.astype(BF16)
    w_p2 = w_p2.astype(BF16)
    ones_bd = ones_bd.astype(BF16)

    in_maps = []
    for c in range(NC_CORES):
        in_maps.append({
            "u_qp": u_qp[c],
            "u_a3": u_a3[c],
            "w_s4": w_s4,
            "w_p2": w_p2,
            "ones_bd": ones_bd,
        })
    return in_maps


# ------------------------------------------------------------- bass builder
def _build_nc():
    from contextlib import ExitStack
    import concourse.bacc as bacc
    import concourse.tile as tile
    from concourse import mybir

    f32 = mybir.dt.float32
    bf16 = mybir.dt.bfloat16
    Act = mybir.ActivationFunctionType
    nc = bacc.Bacc("TRN2", target_bir_lowering=False, debug=False,
                   num_devices=NC_CORES)

    u_qp_p = nc.dram_tensor("u_qp", [128, T16, 2, 128], bf16, kind="ExternalInput")
    u_a3_p = nc.dram_tensor("u_a3", [2, 128, T16, 128], bf16, kind="ExternalInput")
    w_s4_p = nc.dram_tensor("w_s4", [2, 128, C, 128], bf16, kind="ExternalInput")
    w_p2_p = nc.dram_tensor("w_p2", [128, C, 2, M32], bf16, kind="ExternalInput")
    ones_p = nc.dram_tensor("ones_bd", [128, 128], bf16, kind="ExternalInput")
    v_out_p = nc.dram_tensor("v_out", [16, C, BL], f32, kind="ExternalOutput")

    rg = [list(range(NC_CORES))]

    with tile.TileContext(nc) as tc, ExitStack() as ctx:
        sb = ctx.enter_context(tc.tile_pool(name="sb", bufs=1))
        ps = ctx.enter_context(tc.tile_pool(name="ps", bufs=1, space="PSUM"))
        dr = ctx.enter_context(tc.tile_pool(name="dr", bufs=1, space="DRAM"))

        # --- persistent SBUF tiles ---
        u_qp = sb.tile([128, T16, 2, 128], bf16, tag="u_qp")
        u_a3 = [sb.tile([128, T16, 128], bf16, tag=f"u_a3_{k}", name=f"u_a3_{k}") for k in range(2)]
        w_s4 = [sb.tile([128, C, 128], bf16, tag=f"w_s4_{k}", name=f"w_s4_{k}") for k in range(2)]
        w_p2 = sb.tile([128, C, 2, M32], bf16, tag="w_p2")
        ones = sb.tile([128, 128], bf16, tag="ones")
        b_ij = sb.tile([G, T16, 2, C], f32, tag="b_ij")
        cij_bd = sb.tile([128, T16, 2, C], bf16, tag="cij_bd")
        v_bd = sb.tile([128, C, 4, BL], bf16, tag="v_bd")

        # DMA issue order = consumption order: u_qp (q-stage, first), w_s4
        # (s-stage), ones/w_p2 (squash/p), u_a3 last (a-stage, ~30us in)
        for tchunk in range(4):
            nc.sync.dma_start(out=u_qp[:, 4 * tchunk:4 * (tchunk + 1)],
                              in_=u_qp_p[:, 4 * tchunk:4 * (tchunk + 1)])
        for k in range(2):
            nc.sync.dma_start(out=w_s4[k][:], in_=w_s4_p[k])
        nc.sync.dma_start(out=ones[:], in_=ones_p[:])
        nc.sync.dma_start(out=w_p2[:], in_=w_p2_p[:])
        for k in range(2):
            nc.sync.dma_start(out=u_a3[k][:], in_=u_a3_p[k])
        nc.vector.memset(b_ij[:], 0.0)
        # warm-up collective with no input dependency: fires at t~0 so the
        # CC queues initialize while iter-0 compute runs (values unused)
        warm_in = dr.tile([1, 20], f32, tag="warm_in", name="warm_in")
        warm_out = dr.tile([1, 20], f32, tag="warm_out", name="warm_out",
                           addr_space="Shared")
        nc.gpsimd.collective_compute(
            "AllReduce", mybir.AluOpType.add, replica_groups=rg,
            ins=[warm_in[:].opt()], outs=[warm_out[:].opt()])
        nc.vector.memset(cij_bd[:].bitcast(f32), 0.0)
        nc.vector.memset(v_bd[:].bitcast(f32), 0.0)
        eps_c = sb.tile([128, 1], f32, tag="eps_c")
        nc.vector.memset(eps_c[:], 1e-20)
        lnpre = sb.tile([1, 1], f32, tag="lnpre")

        # --- PSUM tiles (persist across iterations) ---
        q_psum = [ps.tile([128, T16, 2, C], f32, tag=f"q_ps{h}", name=f"q_ps{h}") for h in range(2)]
        s_psum = ps.tile([128, C, BL], f32, tag="s_ps")
        mag_ps = ps.tile([128, C, BL], f32, tag="mag_ps")
        p_psum = ps.tile([128, C, 2, M32], f32, tag="p_ps")
        a_psum = ps.tile([128, T16, C, 2], f32, tag="a_ps")

        for it in range(3):
            last = it == 2
            if it == 0:
                # b_ij = 0 -> cij = 0.1 exactly; write the diagonal directly
                for ms in range(2):
                    nc.vector.memset(cij_bd[ms * 64:ms * 64 + G, :, ms, :], 0.1)
                # prefetch the natural_log ACT table during iter-0 q/s
                nc.scalar.activation(lnpre[:], cij_bd[0:1, 0:1, 0:1, 0:1],
                                     Act.Ln, bias=1.0)
            else:
                # ---- softmax over capsules (no max-shift; logits bounded) ----
                eb = sb.tile([G, T16, 2, C], f32, tag="eb")
                ssum = sb.tile([G, T16, 2], f32, tag="ssum")
                rs = sb.tile([G, T16, 2], f32, tag="rs")
                nc.scalar.activation(eb[:], b_ij[:], Act.Exp)
                nc.vector.tensor_reduce(ssum[:], eb[:], axis=mybir.AxisListType.X,
                                        op=mybir.AluOpType.add)
                nc.vector.reciprocal(rs[:], ssum[:])
                for ms in range(2):
                    nc.vector.tensor_mul(
                        cij_bd[ms * 64:ms * 64 + G, :, ms, :],
                        eb[:, :, ms, :],
                        rs[:, :, ms, None].broadcast_to((G, T16, C)))
                # prefetch the natural_log ACT table while the PE runs q/s;
                # input dep on eb pins this after the softmax Exp
                nc.scalar.activation(lnpre[:], eb[0:1, 0:1, 0:1, 0:1], Act.Ln,
                                     bias=1.0)

            # ---- q: per (t, h) matmul, K=128 over (m_sub, g) ----
            for h in range(2):
                for t in range(T16):
                    nc.tensor.matmul(q_psum[h][:, t], lhsT=u_qp[:, t, h, :],
                                     rhs=cij_bd[:, t, :, :], start=True, stop=True)
            # cast PSUM->bf16 (ACT), then transpose 32x32 blocks on DVE; input
            # AP enumerates (c, m): col c*32 + t*2 + ms <- offset t*20 + ms*10 + c
            q_c = [sb.tile([128, T16, 2, C], bf16, tag=f"q_c{h}", name=f"q_c{h}") for h in range(2)]
            q_T = [sb.tile([128, C, M32], bf16, tag=f"q_T{h}", name=f"q_T{h}") for h in range(2)]
            for h in range(2):
                nc.scalar.copy(q_c[h][:], q_psum[h][:])
                src_r = q_c[h][:].rearrange("p t ms c -> p c (t ms)")
                nc.vector.transpose(out=q_T[h][:], in_=src_r)

            # ---- s: per (c, k) accumulate; out rows = (rep, o) 4x-replicated ----
            for c10 in range(C):
                for k in range(2):
                    nc.tensor.matmul(s_psum[:, c10, :], lhsT=w_s4[k][:, c10, :],
                                     rhs=q_T[k][:, c10, :],
                                     start=(k == 0), stop=(k == 1))

            # ---- squash on [64=(rep,o), c, b]: fsc = exp(.5 ln m - ln(1+m)) ----
            s2 = sb.tile([128, C, BL], bf16, tag="s2")
            a_ln = sb.tile([128, C, BL], f32, tag="a_ln")
            b_ln = sb.tile([128, C, BL], f32, tag="b_ln")
            fsc = sb.tile([128, C, BL], f32, tag="fsc")
            v_rep = sb.tile([16, C, BL], bf16, tag="v_rep")
            v_out_sb = sb.tile([16, C, BL], f32, tag="v_out_sb")
            p_sb = sb.tile([128, C, 2, M32], bf16, tag="p_sb")
            # single-chain squash: data deps force the ACT order
            # square -> ln -> ln -> exp, so exactly the Ln and Exp table
            # loads remain (the Ln one prefetched by lnpre above)
            nc.scalar.activation(s2[:], s_psum[:], Act.Square)
            nc.tensor.matmul(mag_ps[:], lhsT=ones[:], rhs=s2[:],
                             start=True, stop=True)
            # pad rows have msq=0: the 1e-20 bias keeps ln finite there
            nc.scalar.activation(a_ln[:], mag_ps[:], Act.Ln, bias=eps_c[:])
            nc.scalar.activation(b_ln[:], mag_ps[:], Act.Ln, bias=1.0)
            # prefetch the Exp table while the DVE runs the stt below
            nc.scalar.activation(lnpre[:], b_ln[0:1, 0:1, 0:1], Act.Exp)
            nc.vector.scalar_tensor_tensor(out=fsc[:],
                                           in0=a_ln[:], scalar=0.5,
                                           in1=b_ln[:],
                                           op0=mybir.AluOpType.mult,
                                           op1=mybir.AluOpType.subtract)
            nc.scalar.activation(fsc[:], fsc[:], Act.Exp)
            if last:
                nc.vector.tensor_mul(v_out_sb[:], s_psum[0:16], fsc[0:16])
            else:
                # v once on DVE (bf16), fan out to the 4 block-diag slots
                # split across DVE and ACT
                nc.vector.tensor_mul(v_rep[:], s_psum[0:16], fsc[0:16])
                for j in range(2):
                    nc.vector.tensor_copy(out=v_bd[j * 32:j * 32 + 16, :, j, :],
                                          in_=v_rep[:])
                for j in range(2, 4):
                    nc.scalar.copy(v_bd[j * 32:j * 32 + 16, :, j, :], v_rep[:])
                for c10 in range(C):
                    nc.tensor.matmul(p_psum[:, c10, :, :],
                                     lhsT=v_bd[:, c10, :, :],
                                     rhs=w_p2[:, c10, :, :],
                                     start=True, stop=True)

            if last:
                nc.sync.dma_start(out=v_out_p[:], in_=v_out_sb[:])
                break

            # k-split casts (ACT + DVE in parallel); the k=0 a-matmul sweep
            # starts as soon as the k=0 cast lands
            nc.scalar.copy(p_sb[:, :, 0, :], p_psum[:, :, 0, :])
            nc.vector.tensor_copy(out=p_sb[:, :, 1, :], in_=p_psum[:, :, 1, :])

            # ---- a: per (t, k) accumulate; out rows = (ms*64+g), cols (c, ms) ----
            for t in range(T16):
                for k in range(2):
                    nc.tensor.matmul(a_psum[:, t, :, :],
                                     lhsT=u_a3[k][:, t, :],
                                     rhs=p_sb[:, :, k, 2 * t:2 * t + 2],
                                     start=(k == 0), stop=(k == 1))
            a_sb = sb.tile([G, T16, 2, C], bf16, tag="a_sb")
            a_red = sb.tile([G, T16, 2, C], bf16, tag="a_red")
            for ms in range(2):
                nc.vector.tensor_copy(out=a_sb[:, :, ms, :],
                                      in_=a_psum[ms * 64:ms * 64 + G, :, :, ms])

            cc_in = dr.tile([G, T16 * 2 * C], bf16, tag=f"cc_in{it}", name=f"cc_in{it}")
            cc_out = dr.tile([G, T16 * 2 * C], bf16, tag=f"cc_out{it}",
                             name=f"cc_out{it}", addr_space="Shared")
            nc.sync.dma_start(out=cc_in[:], in_=a_sb[:])
            nc.gpsimd.collective_compute(
                "AllReduce", mybir.AluOpType.add, replica_groups=rg,
                ins=[cc_in[:].opt()], outs=[cc_out[:].opt()])
            nc.sync.dma_start(out=a_red[:], in_=cc_out[:])
            nc.vector.tensor_add(b_ij[:], b_ij[:], a_red[:])

    nc.finalize()
    return nc


_NC_CACHE = None


def kernel(u, W):
    """u [256,1152,8] f32, W [32,10,16,8] f32 -> [256,10,16,1] f32."""
    global _NC_CACHE
    from concourse import bass_utils

    in_maps = _host_prep(u, W)
    if _NC_CACHE is None:
        _NC_CACHE = _build_nc()
    res = bass_utils.run_bass_kernel_spmd(
        _NC_CACHE, in_maps, core_ids=list(range(NC_CORES)))

    out = np.zeros((NC_CORES * BL, C, O, 1), np.float32)
    for c in range(NC_CORES):
        vo = res.results[c]["v_out"]          # [16, C, BL] = [o, c, b]
        out[c * BL:(c + 1) * BL, :, :, 0] = vo.transpose(2, 1, 0)
    return out


if __name__ == "__main__":
    u = np.random.randn(256, 1152, 8).astype(np.float32)
    W = np.random.randn(32, 10, 16, 8).astype(np.float32)
    v = kernel(u, W)
    print("kernel ran, out shape", v.shape, "absmax", np.abs(v).max())
